# revision 1
# baseline (speedup 1.0000x reference)
"""AdaptiveTripletLoss distributed Trainium2 kernel (8 NeuronCores).

Strategy: shard by class. Host argsorts targets; each class becomes one
128-row padded block. 104 class slots = 13 blocks/core x 8 cores.
Hardest-positive top-3 needs only same-class distances, so each core
computes 13 small 128x128 gram blocks instead of a row-slab of the full
8192x8192 matrix.

v2 structure (vs v1):
- Class centers are computed from RAW embeddings with the per-row 1/norm
  folded into the selection weights (rws = rwm * rcp), so the center
  exchange arms ~10us earlier (no dependency on the full normalize).
- The exchange payload is the pre-transposed, pre-scaled -2*C^T [128,52]
  bf16 tile, sent either via direct peer-to-peer remote DMA broadcasts
  (USE_REMOTE_DMA, XOR-region layout) or the AllGather collective
  (fallback; rank-order layout). Receivers need no center transposes.
- |pos_center|^2 via Q = S_T^T @ negG then diag(Q*S) on DVE (replaces
  the 13 free=512 matmuls + 13 scalar SQUAREs of v1).
- Negative scores as one [104, 13*128] PSUM accumulation (4 fat matmuls)
  with a single broadcast-AP bias add, then 13 transposes + DVE mins.
"""

import numpy as np
from concourse import bacc, mybir, tile, masks
from concourse.bass_types import AP
from concourse.bass_utils import run_bass_kernel_spmd

# Problem constants (hardcoded per harness contract)
N = 8192
D = 512
C = 100
NCORES = 8
BPC = 13              # class blocks per core
NSLOT = BPC * NCORES  # 104 class slots
P = 128               # rows per class block
KCH = D // P          # 4 contraction chunks
CST = 16              # per-chunk column stride (13 used + 3 pad)
FW = KCH * CST        # 64 = free width of the transposed-center tile
BIG = 1.0e4
EPS = 1.0e-12
SENT = 1.0e9          # match_replace sentinel (never present in negG)
REPL = 5.0            # match_replace imm (real negG values are <= ~1)
F32 = mybir.dt.float32
BF16 = mybir.dt.bfloat16

USE_REMOTE_DMA = False
_CACHED_NC = None


def _build_nc():
    nc = bacc.Bacc("TRN2", target_bir_lowering=False, debug=False,
                   num_devices=NCORES)
    emb_h = nc.declare_dram_parameter("emb", [P, BPC * D], BF16, isOutput=False)
    rw_h = nc.declare_dram_parameter("rwm", [P, BPC * BPC], BF16, isOutput=False)
    lw_h = nc.declare_dram_parameter("lw", [P, BPC], F32, isOutput=False)
    pb_h = nc.declare_dram_parameter("padbias", [1, BPC * P], BF16, isOutput=False)
    nb_h = nc.declare_dram_parameter("negbias", [NSLOT, BPC], F32, isOutput=False)
    ic_h = nc.declare_dram_parameter("invc", [BPC, 1], F32, isOutput=False)
    out_h = nc.declare_dram_parameter("out", [P, 1], F32, isOutput=True)

    AX = mybir.AxisListType
    OP = mybir.AluOpType
    AF = mybir.ActivationFunctionType

    with tile.TileContext(nc, num_cores=NCORES) as tc:
        with (
            tc.tile_pool(name="const", bufs=1) as cpool,
            tc.tile_pool(name="big", bufs=1) as bpool,
            tc.tile_pool(name="sm", bufs=1) as spool,
            tc.tile_pool(name="scr", bufs=4) as scr,
            tc.tile_pool(name="gt", bufs=5) as gt,
            tc.tile_pool(name="ps_t", bufs=3, space="PSUM") as ps_t,
            tc.tile_pool(name="ps_a", bufs=3, space="PSUM") as ps_a,
            tc.tile_pool(name="ps_s", bufs=2, space="PSUM") as ps_s,
            tc.tile_pool(name="dram", bufs=1, space="DRAM") as dram,
        ):
            # ---- constants ----
            ident_bf = cpool.tile([P, P], BF16, tag="ident_bf")
            masks.make_identity(nc, ident_bf[:])
            ones = cpool.tile([1, P], BF16, tag="ones")
            nc.vector.memset(ones[:], 1.0)

            # ---- persistent tiles ----
            Eraw = bpool.tile([P, BPC * D], BF16, tag="Eraw")
            Eb = bpool.tile([P, BPC * D], BF16, tag="Eb")
            ETb = bpool.tile([P, BPC * D], BF16, tag="ETb")
            rw_t = spool.tile([P, BPC * BPC], BF16, tag="rw")
            rws = spool.tile([P, BPC * BPC], BF16, tag="rws")
            lw_t = spool.tile([P, BPC], F32, tag="lwt")
            pb_t = spool.tile([1, BPC * P], BF16, tag="pbt")
            nb_t = spool.tile([NSLOT, BPC], F32, tag="nbt")
            ic_t = spool.tile([BPC, 1], F32, tag="ict")
            ssq = spool.tile([P, BPC], F32, tag="ssq")
            nrm = spool.tile([P, BPC], F32, tag="nrm")
            rcp = spool.tile([P, BPC], F32, tag="rcp")
            tsc = spool.tile([P, BPC], F32, tag="tsc")
            a2 = spool.tile([P, BPC], F32, tag="a2")
            dpq = spool.tile([P, BPC], F32, tag="dpq")
            msc = spool.tile([P, BPC], F32, tag="msc")
            dsqs = spool.tile([P, 2 * BPC], F32, tag="dsqs")
            cml = spool.tile([BPC, D], BF16, tag="cml")
            M_l = spool.tile([P, FW], BF16, tag="M_l")
            M_all = spool.tile([P, NCORES * FW], BF16, tag="M_all")
            Mg = spool.tile([P, KCH * NSLOT], BF16, tag="Mg")
            b2c4 = spool.tile([NSLOT, 1], F32, tag="b2c4")
            comb = spool.tile([NSLOT, BPC], F32, tag="comb")
            ssb = spool.tile([NSLOT, BPC * P], BF16, tag="ssb")
            p2q = spool.tile([P, BPC], F32, tag="p2q")
            v3s = spool.tile([P, BPC], F32, tag="v3s")
            v8all = spool.tile([P, 8 * BPC], BF16, tag="v8all")
            negGall = spool.tile([P, BPC * P], BF16, tag="negGall")

            # ---- input DMAs: 4 emb pieces on 4 queues, small on gpsimd ----
            pieces = [(0, 3), (3, 6), (6, 9), (9, BPC)]
            dma_engs = [nc.sync, nc.scalar, nc.gpsimd, nc.sync]
            for eng, (lo, hi) in zip(dma_engs, pieces):
                eng.dma_start(out=Eraw[:, lo * D:hi * D],
                              in_=emb_h[:, lo * D:hi * D])
            nc.scalar.dma_start(out=rw_t[:], in_=rw_h[:])
            nc.gpsimd.dma_start(out=lw_t[:], in_=lw_h[:])
            nc.gpsimd.dma_start(out=pb_t[:], in_=pb_h[:])
            nc.scalar.dma_start(out=nb_t[:], in_=nb_h[:])
            nc.gpsimd.dma_start(out=ic_t[:], in_=ic_h[:])

            # ---- W1: row sum-of-squares wave ----
            for b in range(BPC):
                bsl = slice(b * D, (b + 1) * D)
                sq = scr.tile([P, D], F32, tag="sq")
                if b % 3 != 1:
                    nc.vector.scalar_tensor_tensor(
                        sq[:], in0=Eraw[:, bsl], scalar=1.0, in1=Eraw[:, bsl],
                        op0=OP.mult, op1=OP.mult, accum_out=ssq[:, b:b + 1])
                else:
                    nc.scalar.activation(sq[:], Eraw[:, bsl], AF.Square,
                                         accum_out=ssq[:, b:b + 1])
            # ---- W2: norm scalars, grouped by DMA piece ----
            for lo, hi in pieces:
                nc.scalar.activation(nrm[:, lo:hi], ssq[:, lo:hi], AF.Sqrt)
                nc.vector.tensor_scalar_max(nrm[:, lo:hi], nrm[:, lo:hi], EPS)
                nc.vector.reciprocal(rcp[:, lo:hi], nrm[:, lo:hi])
                # rws block cols: rwm * rcp (per-block broadcast along free)
                rc = rcp[:]
                rcb = AP(rc.tensor, rc.offset + lo,
                         [rc.ap[0], [1, hi - lo], [0, BPC]])
                rwv = rw_t[:]
                rww = AP(rwv.tensor, rwv.offset + lo * BPC,
                         [rwv.ap[0], [BPC, hi - lo], [1, BPC]])
                rsv = rws[:]
                rsw = AP(rsv.tensor, rsv.offset + lo * BPC,
                         [rsv.ap[0], [BPC, hi - lo], [1, BPC]])
                nc.vector.tensor_tensor(rsw, rww, rcb, op=OP.mult)
            nc.vector.tensor_mul(tsc[:], rcp[:], rcp[:])
            nc.vector.tensor_mul(a2[:], ssq[:], tsc[:])

            # ---- W4': class centers from RAW rows, then -2*C^T and exchange ----
            pcn = ps_a.tile([BPC, D], F32, tag="pa")
            for b in range(BPC):
                nc.tensor.matmul(pcn[:], lhsT=rws[:, b * BPC:(b + 1) * BPC],
                                 rhs=Eraw[:, b * D:(b + 1) * D],
                                 start=(b == 0), stop=(b == BPC - 1))
            # cml = -2 * (1/cnt) * pcn   [13, 512] bf16
            nc.vector.tensor_scalar(cml[:], pcn[:], ic_t[:], -2.0,
                                    op0=OP.mult, op1=OP.mult)
            tpm = ps_t.tile([P, FW], BF16, tag="pt")
            for k in range(KCH):
                nc.tensor.transpose(tpm[:, k * CST:k * CST + BPC],
                                    cml[:, k * P:(k + 1) * P],
                                    ident_bf[0:BPC, 0:BPC])
            nc.vector.tensor_copy(M_l[:], tpm[:])

            with (
                nc.semaphore("rdb_rsem") as rsem,
                nc.semaphore("rdb_lsem") as lsem,
            ):
                if USE_REMOTE_DMA:
                    # Region r on every receiver holds sender (self ^ r)'s
                    # -2*C^T tile; host compensates the order in negbias.
                    for r in range(NCORES):
                        rdests = [None] * NCORES
                        rdests[r] = (0, r)
                        nc.gpsimd.remote_dma_broadcast(
                            out_ap=M_all[:, r * FW:(r + 1) * FW],
                            in_ap=M_l[:],
                            remote_sem=rsem,
                            local_sem=lsem,
                            rdests=rdests,
                        )
                    nc.gpsimd.trigger_dma(count=None)
                else:
                    cc_in = dram.tile([P, FW], BF16, tag="cc_in")
                    cc_out = dram.tile([NCORES * P, FW], BF16,
                                       addr_space="Shared", tag="cc_out")
                    nc.sync.dma_start(out=cc_in[:], in_=M_l[:])
                    nc.gpsimd.collective_compute(
                        "AllGather", OP.bypass,
                        replica_groups=[list(range(NCORES))],
                        ins=[cc_in[:].opt()],
                        outs=[cc_out[:].opt()],
                    )
                    ccv = cc_out[:]
                    cc3 = AP(ccv.tensor, ccv.offset,
                             [[FW, P], [P * FW, NCORES], [1, FW]])
                    nc.sync.dma_start(out=M_all[:], in_=cc3)

                # ---- W3: scale to unit rows (bf16), alternating engines ----
                for b in range(BPC):
                    bsl = slice(b * D, (b + 1) * D)
                    if b % 3 == 1:
                        nc.scalar.activation(Eb[:, bsl], Eraw[:, bsl], AF.Copy,
                                             scale=rcp[:, b:b + 1])
                    else:
                        nc.vector.tensor_scalar(Eb[:, bsl], Eraw[:, bsl],
                                                rcp[:, b:b + 1], None,
                                                op0=OP.mult)
                # ---- W5: transpose wave ----
                for b in range(BPC):
                    pt = ps_t.tile([P, D], BF16, tag="pt")
                    for k in range(KCH):
                        nc.tensor.transpose(
                            pt[:, k * P:(k + 1) * P],
                            Eb[:, b * D + k * P:b * D + (k + 1) * P],
                            ident_bf[:])
                    if b % 2 == 0:
                        nc.vector.tensor_copy(ETb[:, b * D:(b + 1) * D], pt[:])
                    else:
                        nc.scalar.activation(ETb[:, b * D:(b + 1) * D], pt[:],
                                             AF.Copy)

                # ---- W6: gram wave + negate (bf16) ----
                for b in range(BPC):
                    pg = ps_a.tile([P, P], F32, tag="pa")
                    for k in range(KCH):
                        sl = slice(b * D + k * P, b * D + (k + 1) * P)
                        nc.tensor.matmul(pg[:], lhsT=ETb[:, sl], rhs=ETb[:, sl],
                                         start=(k == 0), stop=False)
                    nc.tensor.matmul(pg[:], lhsT=ones[:],
                                     rhs=pb_t[0:1, b * P:(b + 1) * P],
                                     start=False, stop=True)
                    if b % 2 == 0:
                        nc.vector.tensor_scalar_mul(
                            negGall[:, b * P:(b + 1) * P], pg[:], -1.0)
                    else:
                        nc.scalar.activation(negGall[:, b * P:(b + 1) * P],
                                             pg[:], AF.Copy, scale=-1.0)

                # ---- W7+W8': top-3 select; p2q = sum of selected negG pairs ----
                for b in range(BPC):
                    negG = negGall[:, b * P:(b + 1) * P]
                    v8 = v8all[:, 8 * b:8 * b + 8]
                    nc.vector.max(v8, negG[:])
                    nc.vector.memset(v8all[:, 8 * b + 3:8 * b + 8], SENT)
                    Gm = gt.tile([P, P], BF16, tag="Gm")
                    nc.vector.match_replace(Gm[:], v8, negG, REPL)
                    Sb = gt.tile([P, P], BF16, tag="Sb")
                    nc.vector.tensor_scalar(Sb[:], Gm[:], REPL - 1.0, None,
                                            op0=OP.is_ge)
                    pst = ps_t.tile([P, P], BF16, tag="pt")
                    nc.tensor.transpose(pst[:], Sb[:], ident_bf[:])
                    S_T = gt.tile([P, P], BF16, tag="S_T")
                    if b % 2 == 0:
                        nc.vector.tensor_copy(S_T[:], pst[:])
                    else:
                        nc.scalar.activation(S_T[:], pst[:], AF.Copy)
                    Qp = ps_a.tile([P, P], F32, tag="pa")
                    nc.tensor.matmul(Qp[:], lhsT=S_T[:], rhs=negG,
                                     start=True, stop=True)
                    qs = scr.tile([P, P], BF16, tag="qs")
                    nc.vector.scalar_tensor_tensor(
                        qs[:], in0=Qp[:], scalar=1.0, in1=Sb[:],
                        op0=OP.mult, op1=OP.mult, accum_out=p2q[:, b:b + 1])

                # ---- d_pos^2 = a2 + (2/3)*v3s - p2q/9 ----
                v8v = v8all[:]
                v8_3 = AP(v8v.tensor, v8v.offset,
                          [v8v.ap[0], [8, BPC], [1, 3]])
                nc.vector.tensor_reduce(v3s[:], v8_3, axis=AX.X, op=OP.add)
                t2a = spool.tile([P, BPC], F32, tag="t2a")
                nc.vector.tensor_scalar(t2a[:], p2q[:], -1.0 / 9.0, None,
                                        op0=OP.mult)
                nc.vector.scalar_tensor_tensor(dpq[:], in0=v3s[:],
                                               scalar=2.0 / 3.0, in1=t2a[:],
                                               op0=OP.mult, op1=OP.add)
                nc.vector.tensor_tensor(dsqs[:, 0:BPC], dpq[:], a2[:],
                                        op=OP.add)
                dsc = spool.tile([P, 2 * BPC], F32, tag="dsc")
                ds = spool.tile([P, 2 * BPC], F32, tag="ds")
                nc.vector.tensor_scalar_max(dsc[:, 0:BPC], dsqs[:, 0:BPC], EPS)
                nc.scalar.activation(ds[:, 0:BPC], dsc[:, 0:BPC], AF.Sqrt)

                # ---- tail: gather M, b2, scores, min over centers ----
                tc.tile_set_cur_wait(5.0)
                if USE_REMOTE_DMA:
                    with tc.tile_critical():
                        nc.gpsimd.wait_ge(rsem, 16)
                        nc.gpsimd.wait_ge(lsem, 128)
                mav = M_all[:]
                for k in range(KCH):
                    mview = AP(mav.tensor, mav.offset + k * CST,
                               [mav.ap[0], [FW, NCORES], [1, BPC]])
                    nc.vector.tensor_copy(
                        Mg[:, k * NSLOT:(k + 1) * NSLOT], mview)
                # b2*4 = diag(M^T M) via 4 matmuls + identity-masked accum
                mtm = ps_t.tile([NSLOT, NSLOT], F32, tag="pt")
                for k in range(KCH):
                    mgk = Mg[:, k * NSLOT:(k + 1) * NSLOT]
                    nc.tensor.matmul(mtm[:], lhsT=mgk, rhs=mgk,
                                     start=(k == 0), stop=(k == KCH - 1))
                dsc4 = scr.tile([NSLOT, NSLOT], BF16, tag="sq")
                nc.vector.scalar_tensor_tensor(
                    dsc4[:], in0=mtm[:], scalar=1.0,
                    in1=ident_bf[0:NSLOT, 0:NSLOT],
                    op0=OP.mult, op1=OP.mult, accum_out=b2c4[:])
                bcv = b2c4[:]
                bcb = AP(bcv.tensor, bcv.offset, [bcv.ap[0], [0, BPC]])
                nc.vector.scalar_tensor_tensor(comb[:], in0=bcb, scalar=0.25,
                                               in1=nb_t[:], op0=OP.mult,
                                               op1=OP.add)
                # scores: [104, nb*128] PSUM groups (<=512 f32 per bank)
                groups = [(0, 4), (4, 8), (8, 12), (12, BPC)]
                etv = ETb[:]
                for lo, hi in groups:
                    nb = hi - lo
                    sc = ps_s.tile([NSLOT, 4 * P], F32, tag="sc")
                    for k in range(KCH):
                        rview = AP(etv.tensor, etv.offset + lo * D + k * P,
                                   [etv.ap[0], [D, nb], [1, P]])
                        nc.tensor.matmul(
                            sc[:, 0:nb * P],
                            lhsT=Mg[:, k * NSLOT:(k + 1) * NSLOT],
                            rhs=rview, start=(k == 0), stop=(k == KCH - 1))
                    # bias add with per-block broadcast of comb, out bf16
                    scv = sc[:]
                    sc3 = AP(scv.tensor, scv.offset,
                             [scv.ap[0], [P, nb], [1, P]])
                    ssv = ssb[:]
                    ss3 = AP(ssv.tensor, ssv.offset + lo * P,
                             [ssv.ap[0], [P, nb], [1, P]])
                    cbv = comb[:]
                    cb3 = AP(cbv.tensor, cbv.offset + lo,
                             [cbv.ap[0], [1, nb], [0, P]])
                    nc.vector.tensor_tensor(ss3, sc3, cb3, op=OP.add)
                xs = spool.tile([P, BPC], F32, tag="xs")
                for lo, hi in groups:
                    for j in range(lo, hi):
                        pts = ps_t.tile([P, NSLOT], BF16, tag="pt")
                        nc.tensor.transpose(pts[:], ssb[:, j * P:(j + 1) * P],
                                            ident_bf[0:NSLOT, 0:NSLOT])
                        nc.vector.tensor_reduce(msc[:, j:j + 1], pts[:],
                                                axis=AX.X, op=OP.min)
                    gs = slice(BPC + lo, BPC + hi)
                    nc.vector.tensor_tensor(dsqs[:, gs], msc[:, lo:hi],
                                            a2[:, lo:hi], op=OP.add)
                    nc.vector.tensor_scalar_max(dsc[:, gs], dsqs[:, gs], EPS)
                    nc.scalar.activation(ds[:, gs], dsc[:, gs], AF.Sqrt)
                    nc.vector.tensor_sub(xs[:, lo:hi], ds[:, lo:hi],
                                         ds[:, gs])

                # ---- loss tail ----
                # softplus(x) = 0.5x + h(x^2), deg-5 poly in u=x^2
                PC = [6.931485008076e-01, 1.249840895147e-01,
                      -5.177011703000e-03, 3.240810187699e-04,
                      -1.812813478166e-05, 5.616111839003e-07]
                uq = spool.tile([P, BPC], F32, tag="uq")
                nc.vector.tensor_mul(uq[:], xs[:], xs[:])
                ph = spool.tile([P, BPC], F32, tag="ph")
                nc.vector.tensor_scalar_mul(ph[:], uq[:], PC[5])
                for k in (4, 3, 2, 1):
                    nc.vector.scalar_tensor_tensor(ph[:], in0=ph[:],
                                                   scalar=PC[k], in1=uq[:],
                                                   op0=OP.add, op1=OP.mult)
                lp = spool.tile([P, BPC], F32, tag="lp")
                nc.vector.scalar_tensor_tensor(lp[:], in0=xs[:], scalar=0.5,
                                               in1=ph[:], op0=OP.mult,
                                               op1=OP.add)
                nc.vector.tensor_scalar_add(lp[:], lp[:], PC[0])
                wl = spool.tile([P, BPC], F32, tag="wl")
                accrow = spool.tile([P, 1], F32, tag="accrow")
                nc.vector.scalar_tensor_tensor(wl[:], in0=lp[:], scalar=1.0,
                                               in1=lw_t[:], op0=OP.mult,
                                               op1=OP.mult, accum_out=accrow[:])
                nc.sync.dma_start(out=out_h[:], in_=accrow[:])

    nc.finalize()
    return nc


def _get_nc():
    global _CACHED_NC
    if _CACHED_NC is None:
        _CACHED_NC = _build_nc()
    return _CACHED_NC


def _prep_inputs(embeddings, targets):
    """Host-side sharding: class-sorted, padded to 128-row class blocks."""
    import ml_dtypes
    emb = np.ascontiguousarray(np.asarray(embeddings, dtype=np.float32))
    tgt = np.asarray(targets).astype(np.int64)
    counts = np.bincount(tgt, minlength=C)
    if counts.max() > P:
        raise ValueError(f"class count {counts.max()} exceeds block size {P}")
    order = np.argsort(tgt, kind="stable")
    offs = np.zeros(C + 1, dtype=np.int64)
    np.cumsum(counts, out=offs[1:])

    emb_pad = np.zeros((NCORES, P, BPC * D), dtype=ml_dtypes.bfloat16)
    rwm = np.zeros((NCORES, P, BPC * BPC), dtype=ml_dtypes.bfloat16)
    lw = np.zeros((NCORES, P, BPC), dtype=np.float32)
    padbias = np.zeros((NCORES, 1, BPC * P), dtype=ml_dtypes.bfloat16)
    negbias = np.zeros((NCORES, NSLOT, BPC), dtype=np.float32)
    invc = np.zeros((NCORES, BPC, 1), dtype=np.float32)

    for slot in range(NSLOT):
        core, b = slot // BPC, slot % BPC
        if slot < C:
            cnt = int(counts[slot])
            rows = order[offs[slot]:offs[slot] + cnt]
            emb_pad[core, :cnt, b * D:(b + 1) * D] = emb[rows]
        else:
            cnt = 0
        padbias[core, 0, b * P + cnt:(b + 1) * P] = BIG
        if cnt:
            rwm[core, :cnt, b * BPC + b] = 1.0
            invc[core, b, 0] = 1.0 / cnt
            if cnt >= 2:
                lw[core, :cnt, b] = 1.0
        # exclude own class and empty/pad class slots from the negative min.
        # Position pos=13r+j in M_all on core `core` holds the centers of
        # sender s (remote: s = core^r; collective: s = r), class 13s+j.
        for pos in range(NSLOT):
            r, j = pos // BPC, pos % BPC
            s = (core ^ r) if USE_REMOTE_DMA else r
            c = BPC * s + j
            if c == slot or c >= C or counts[c] == 0:
                negbias[core, pos, b] = BIG

    denom = float(counts[counts >= 2].sum())
    return emb_pad, rwm, lw, padbias, negbias, invc, denom


def _make_in_maps(emb_pad, rwm, lw, padbias, negbias, invc):
    return [
        {
            "emb": emb_pad[i],
            "rwm": rwm[i],
            "lw": lw[i],
            "padbias": padbias[i],
            "negbias": negbias[i],
            "invc": invc[i],
        }
        for i in range(NCORES)
    ]


def kernel(embeddings, targets, num_classes):
    import time
    emb_pad, rwm, lw, padbias, negbias, invc, denom = _prep_inputs(
        embeddings, targets)
    nc = _get_nc()
    in_maps = _make_in_maps(emb_pad, rwm, lw, padbias, negbias, invc)
    res = None
    for attempt in range(3):
        try:
            res = run_bass_kernel_spmd(nc, in_maps, core_ids=list(range(NCORES)))
            break
        except Exception:
            # transient device wedges (NRT_EXEC_UNIT_UNRECOVERABLE) clear
            # after a cooldown; retry rather than failing the whole call
            if attempt == 2:
                raise
            time.sleep(45)
    parts = [np.asarray(res.results[i]["out"], dtype=np.float64).sum() for i in range(NCORES)]
    loss = np.float32(np.sum(np.asarray(parts, dtype=np.float64)) / max(denom, 1.0))
    return np.asarray(loss, dtype=np.float32)

